# revision 4
# baseline (speedup 1.0000x reference)
"""FISTA encoder v2: exact-class fp16 3-term matmuls + fused custom shrink.

A@x decomposed as Ah@xh + Ah@xl + Al@xh with fp16 Dekker-style pairs
(dropped Al@xl ~ 2^-22). Static Y folded into the contraction with a
3-way fp16 split (Yh/Ym/Yl) riding existing matmul slots, so the static
DtY part is exact to ~2^-33. x is kept fp32 only transiently per group;
matmul state is the (xh, xl) fp16 pair. u_n = A@x_n + DtY accumulates in
PSUM; z_n = (1+tt)*u_n - v_{n-1}; x_{n+1} = softshrink(z_n) in one fused
DVE op; v_n = tt_n*u_n saved by ACT.
"""
import sys
if '/opt/trn_rl_repo' not in sys.path:
    sys.path.insert(0, '/opt/trn_rl_repo')
import numpy as np

# ---- custom fused DVE op: x = softshrink(C0*u - v, lambd) ----------------
def _register_shrink_op():
    from concourse.dve_ops import OPS, DveOp
    from concourse import dve_ops as _d
    from concourse.dve_spec import Spec, Src0, Src1, C0, C1, Zero, maxx, minn
    for op in OPS:
        if op.name == "FISTA_SHRINK":
            return op
    _q = Src0 * C0 - Src1
    _c = minn(maxx(_q, Zero - C1), C1)

    def _ref(in0, in1, s0, s1, imm2):
        q = in0 * s0 - in1
        return q - np.clip(q, -s1, s1)

    op = DveOp("FISTA_SHRINK", Spec(body=_q - _c, reference=_ref),
               subdim=False, uops_sha={})
    OPS.append(op)
    _d._SUB_OPCODE_FOR_NAME[op.name] = _d._CUSTOM_DVE_ROW_BASE + len(OPS) - 1
    _d.CUSTOM_DVE_SPECS[op.name] = op.spec
    for ver in ("v3", "v4"):
        try:
            op.compile(ver)
        except ValueError as e:
            got = str(e).split(f"{ver}: ")[1].split(" ")[0]
            op.uops_sha[ver] = got
            op.compile(ver)
    return op


def _shrink(nc, out, u, v, c0, lambd):
    op = _register_shrink_op()
    return nc.vector._custom_dve(op, out=out, in0=u, in1=v,
                                 s0=float(c0), s1=float(lambd))


T = 36
MAXITER = 100
LAMBD0 = 0.01
N_CORES = 8
B, P, K = 4, 16384, 161
PLOC = P // N_CORES
F = B * PLOC                 # 8192 columns per core
NG = 8
FG = F // NG                 # 1024 columns per group

_CACHE = {}


def _host_constants(Drr, Dtheta):
    Drr = np.asarray(Drr, np.float32)
    Dtheta = np.asarray(Dtheta, np.float32)
    i = np.arange(T, dtype=np.float32)
    powr = (Drr[None, :] ** i[:, None]).astype(np.float32)
    sign = np.where(i[:, None] % 2 == 0, np.float32(1.0), np.float32(-1.0))
    ang = (i[:, None] * Dtheta[None, :]).astype(np.float32)
    cosm = np.cos(ang).astype(np.float32)
    sinm = np.sin(ang).astype(np.float32)
    ones = np.ones((T, 1), np.float32)
    dic = np.concatenate(
        [ones, powr * cosm, sign * powr * cosm, powr * sinm, sign * powr * sinm],
        axis=1).astype(np.float32)
    G = np.sqrt(np.sum(dic * dic, axis=0, dtype=np.float32)).astype(np.float32)
    G = np.where(G == 0, np.sqrt(np.float32(T)), G).astype(np.float32)
    D = (dic / G).astype(np.float32)
    DtD = (D.T @ D).astype(np.float32)
    L = np.sqrt(np.sum(DtD.astype(np.float64) ** 2)).astype(np.float32)
    linv = np.float32(1.0) / L
    A = (np.eye(K, dtype=np.float32) - DtD * linv).astype(np.float32)
    lambd = np.float32(LAMBD0 * linv)
    W = np.concatenate([A, (D * linv).astype(np.float32)], axis=0)  # (197,161)

    Wh = W.astype(np.float16)
    Wl = (W - Wh.astype(np.float32)).astype(np.float16)
    wh1 = np.ascontiguousarray(Wh[0:128])          # (128,161) x_hi rows, hi part
    wl1 = np.ascontiguousarray(Wl[0:128])
    # S_A rows: [xh_lo(33); Yh(36); Ym(36)] -> T1 weights [Ah_lo; Dh; Dh]
    wa1 = np.concatenate([Wh[128:161], Wh[161:197], Wh[161:197]], axis=0)  # (105,161)
    # S_A x T3 weights [Al_lo; Dl; Dl]
    wa2 = np.concatenate([Wl[128:161], Wl[161:197], Wl[161:197]], axis=0)  # (105,161)
    # S_B rows: [xl_lo(33); Yl(36)] -> weights [Ah_lo; Dh]
    wb = np.concatenate([Wh[128:161], Wh[161:197]], axis=0)                # (69,161)

    tts = []
    t = np.float32(1.0)
    for _ in range(MAXITER):
        t_new = (np.float32(1.0) + np.sqrt(np.float32(1.0) + np.float32(4.0) * t * t)) / np.float32(2.0)
        tts.append(np.float32((t - np.float32(1.0)) / t_new))
        t = t_new
    return dict(wh1=wh1, wl1=wl1, wa1=wa1, wa2=wa2, wb=wb,
                lambd=lambd, tts=tts)


def _build_bass(lambd, tts, xl_engine="gpsimd"):
    import concourse.bass as bass
    import concourse.tile as tile
    from concourse import bacc, mybir
    dt = mybir.dt

    nc = bacc.Bacc("TRN2", target_bir_lowering=False, debug=False,
                   num_devices=N_CORES)
    wh1_d = nc.dram_tensor("wh1", [128, K], dt.float16, kind="ExternalInput").ap()
    wl1_d = nc.dram_tensor("wl1", [128, K], dt.float16, kind="ExternalInput").ap()
    wa1_d = nc.dram_tensor("wa1", [105, K], dt.float16, kind="ExternalInput").ap()
    wa2_d = nc.dram_tensor("wa2", [105, K], dt.float16, kind="ExternalInput").ap()
    wb_d = nc.dram_tensor("wb", [69, K], dt.float16, kind="ExternalInput").ap()
    yh_d = nc.dram_tensor("yh", [T, F], dt.float16, kind="ExternalInput").ap()
    ym_d = nc.dram_tensor("ym", [T, F], dt.float16, kind="ExternalInput").ap()
    yl_d = nc.dram_tensor("yl", [T, F], dt.float16, kind="ExternalInput").ap()
    xout_d = nc.dram_tensor("xout", [K, F], dt.float32, kind="ExternalOutput").ap()

    lam = float(lambd)

    with tile.TileContext(nc) as tc:
        with tc.tile_pool(name="wp", bufs=1) as wp, \
             tc.tile_pool(name="state", bufs=1) as state, \
             tc.tile_pool(name="xt", bufs=3) as xtp, \
             tc.tile_pool(name="ph", bufs=2, space="PSUM") as php, \
             tc.tile_pool(name="pl", bufs=2, space="PSUM") as plp:

            wh1 = wp.tile([128, K], dt.float16, tag="wh1")
            wl1 = wp.tile([128, K], dt.float16, tag="wl1")
            wa1 = wp.tile([105, K], dt.float16, tag="wa1")
            wa2 = wp.tile([105, K], dt.float16, tag="wa2")
            wb = wp.tile([69, K], dt.float16, tag="wb")
            for t_, d_ in ((wh1, wh1_d), (wl1, wl1_d), (wa1, wa1_d),
                           (wa2, wa2_d), (wb, wb_d)):
                nc.sync.dma_start(t_[:], d_[:])

            XH1, XL1, SA, SB, V1, V2 = [], [], [], [], [], []
            for g in range(NG):
                cols = slice(g * FG, (g + 1) * FG)
                xh1 = state.tile([128, FG], dt.float16, tag=f"xh1_{g}")
                xl1 = state.tile([128, FG], dt.float16, tag=f"xl1_{g}")
                sa = state.tile([105, FG], dt.float16, tag=f"sa_{g}")
                sb_ = state.tile([69, FG], dt.float16, tag=f"sb_{g}")
                v1 = state.tile([128, FG], dt.float32, tag=f"v1_{g}")
                v2 = state.tile([33, FG], dt.float32, tag=f"v2_{g}")
                nc.vector.memset(xh1[:], 0.0)
                nc.vector.memset(xl1[:], 0.0)
                nc.vector.memset(sa[0:33, :], 0.0)
                nc.vector.memset(sb_[0:33, :], 0.0)
                nc.vector.memset(v1[:], 0.0)
                nc.vector.memset(v2[:], 0.0)
                nc.sync.dma_start(sa[33:69, :], yh_d[:, cols])
                nc.sync.dma_start(sa[69:105, :], ym_d[:, cols])
                nc.sync.dma_start(sb_[33:69, :], yl_d[:, cols])
                XH1.append(xh1); XL1.append(xl1); SA.append(sa); SB.append(sb_)
                V1.append(v1); V2.append(v2)

            xl_eng = nc.gpsimd if xl_engine == "gpsimd" else nc.vector

            for n in range(MAXITER):
                c0 = float(np.float32(1.0) + (tts[n - 1] if n > 0 else np.float32(0.0)))
                sv = float(tts[n])
                last = (n == MAXITER - 1)
                for g in range(NG):
                    xh1, xl1, sa, sb_ = XH1[g], XL1[g], SA[g], SB[g]
                    v1, v2 = V1[g], V2[g]
                    ph = php.tile([128, FG], dt.float32, tag="ph")
                    pl = plp.tile([33, FG], dt.float32, tag="pl")
                    # weight-major order: each weight block loads once and
                    # serves both 512-col halves before switching.
                    if n == 0:
                        mm_list = [(wa1, sa), (wb, sb_), (wa2, sa)]
                    else:
                        mm_list = [(wh1, xh1), (wh1, xl1), (wl1, xh1),
                                   (wa1, sa), (wb, sb_), (wa2, sa)]
                    nmm = len(mm_list)
                    for pt, wlo, whi in ((ph, 0, 128), (pl, 128, K)):
                        wc = slice(wlo, whi)
                        for mi, (wt_, rt_) in enumerate(mm_list):
                            for h in range(FG // 512):
                                s = slice(h * 512, (h + 1) * 512)
                                nc.tensor.matmul(pt[:, s], wt_[:, wc], rt_[:, s],
                                                 start=(mi == 0),
                                                 stop=(mi == nmm - 1))
                    # x fp32, transient
                    x1t = xtp.tile([128, FG], dt.float32, tag="x1t")
                    x2t = xtp.tile([33, FG], dt.float32, tag="x2t")
                    _shrink(nc, x1t[:], ph[:], v1[:], c0, lam)
                    _shrink(nc, x2t[:], pl[:], v2[:], c0, lam)
                    if not last:
                        nc.scalar.mul(v1[:], ph[:], sv)
                        nc.scalar.mul(v2[:], pl[:], sv)
                        # xh = fp16(x); xl = x - xh
                        nc.scalar.copy(xh1[:], x1t[:])
                        nc.scalar.copy(sa[0:33, :], x2t[:])
                        xl_eng.tensor_sub(xl1[:], x1t[:], xh1[:])
                        xl_eng.tensor_sub(sb_[0:33, :], x2t[:], sa[0:33, :])
                    else:
                        cols = slice(g * FG, (g + 1) * FG)
                        nc.sync.dma_start(xout_d[0:128, cols], x1t[:])
                        nc.sync.dma_start(xout_d[128:K, cols], x2t[:])

    nc.compile()
    return nc


def _get_compiled(Drr, Dtheta):
    key = (np.asarray(Drr, np.float32).tobytes(),
           np.asarray(Dtheta, np.float32).tobytes())
    if key not in _CACHE:
        hc = _host_constants(Drr, Dtheta)
        nc = _build_bass(hc["lambd"], hc["tts"])
        _CACHE[key] = (nc, hc)
    return _CACHE[key]


def kernel(x, Drr, Dtheta):
    from concourse.bass_utils import run_bass_kernel_spmd
    x = np.asarray(x, np.float32)
    nc, hc = _get_compiled(Drr, Dtheta)

    in_maps = []
    for c in range(N_CORES):
        xs = x[:, :, c * PLOC:(c + 1) * PLOC]
        yfull = np.ascontiguousarray(xs.transpose(1, 0, 2).reshape(T, F))
        yh = yfull.astype(np.float16)
        ym = (yfull - yh.astype(np.float32)).astype(np.float16)
        yl = (yfull - yh.astype(np.float32) - ym.astype(np.float32)).astype(np.float16)
        in_maps.append({"wh1": hc["wh1"], "wl1": hc["wl1"], "wa1": hc["wa1"],
                        "wa2": hc["wa2"], "wb": hc["wb"],
                        "yh": yh, "ym": ym, "yl": yl})

    res = run_bass_kernel_spmd(nc, in_maps, core_ids=list(range(N_CORES)))
    global LAST_RESULTS
    LAST_RESULTS = res
    out = np.empty((B, K, P), np.float32)
    for c in range(N_CORES):
        xo = res.results[c]["xout"]
        out[:, :, c * PLOC:(c + 1) * PLOC] = (
            xo.reshape(K, B, PLOC).transpose(1, 0, 2))
    return out
